# revision 1
# baseline (speedup 1.0000x reference)
"""MatchingNet model kernel for 8 Trainium2 NeuronCores.

Computation (reference semantics, N=4096, E=512, G=256, V=50000, R=1000):
  x  = embedding[input]          (N, E)
  ex = embedding[set_inputs]     (2, N, E)
  g_out = bidirectional 2-step LSTM over ex   (2, N, E)
  fh = lstm_f(x) + x             (N, E)          [single step, zero state]
  scores[b] = g_out[b] @ fh.T    (2, N, N)
  a = softmax(scores, axis=0)    -- softmax over b (size 2), pointwise in (n,m)
  r[b] = a[b] @ g_out[b]         (2, N, E)
  dot/nr/ng reductions over n -> cos (2, E) -> tiny tail -> softmax (R,)

Sharding: data-parallel over N. Core k owns rows [512k, 512k+512).
Each core runs the LSTMs for its own rows, then:
  AG1 all-gathers fh.T (keys), AG2 all-gathers g0/g1 (values).
  Attention trick: a[0] = sigmoid(D), a[1] = 1 - a[0] with
  D = (g0 - g1) @ fh.T, so only 3 N*N*E matmuls instead of 4.
Each core computes its n-shard of D and r, reduces over its n rows to
partials dot/sr/sg (2, E); the host sums partials and runs the O(R*E)
tail (cosine normalize, projection collapse, final softmax).
"""

import os
import sys

import numpy as np

for _p in ("/opt/trn_rl_repo", os.path.expanduser("~/.axon_site/_ro/trn_rl_repo")):
    if os.path.isdir(_p) and _p not in sys.path:
        sys.path.insert(0, _p)

import concourse.bacc as bacc
import concourse.bass as bass
import concourse.mybir as mybir
import concourse.tile as tile
from concourse import bass_utils
from concourse.masks import make_identity

N, E, G, V, R = 4096, 512, 256, 50000, 1000
NCORES = 8
NL = N // NCORES  # 512 rows per core
P = 128
NE = E // P   # 4 e-chunks
NH = G // P   # 2 hidden chunks for the g-LSTM
EPS = 1e-8

F32 = mybir.dt.float32
F32R = mybir.dt.float32r
I32 = mybir.dt.int32
AF = mybir.ActivationFunctionType
ALU = mybir.AluOpType


def _mm(ap):
    """Matmul operands are stored as float32r tiles already."""
    return ap


def _lstm_cell(nc, pools, H, xT, W_sb, U_sb, hprevT, cprevT, bias_sb, h_out, c_out):
    """Emit one LSTM cell, transposed layout (feature on partition, n free).

    gates.T[j, n] = sum_e W.T[e, j] x.T[e, n] (+ sum_h U.T[h, j] hprev.T[h, n]) + b[j]
    xT: (P, 4, NL); W_sb: (P, 4, 4H); U_sb: (P, H//P, 4H) or None.
    h_out/c_out: (P, H//P, NL). When cprevT is None the forget gate is skipped
    (sigmoid(f) * 0 contributes nothing) and c = sig(i)*tanh(g).
    """
    pg, gp, tp = pools["pg"], pools["gates"], pools["tmp"]
    hc = H // P
    nj = 4 * H // P
    gb = [gp.tile([P, hc, NL], F32, tag=f"gate{g}", bufs=1, name=f"gb{g}")
          for g in range(4)]
    if c_out is None:
        assert cprevT is None
        c_out = gb[1]  # forget-gate buffer is unused for zero-state cells
    for jc in range(nj):
        g = jc // hc
        if cprevT is None and g == 1:
            continue  # forget gate unused with zero initial state
        ps = pg.tile([P, NL], F32, tag="pg", bufs=4, name="ps_gate")
        js = slice(jc * P, (jc + 1) * P)
        for kt in range(NE):
            nc.tensor.matmul(
                ps[:], _mm(W_sb[:, kt, js]), _mm(xT[:, kt, :]),
                start=(kt == 0), stop=(U_sb is None and kt == NE - 1))
        if U_sb is not None:
            for kt in range(hc):
                nc.tensor.matmul(
                    ps[:], _mm(U_sb[:, kt, js]), _mm(hprevT[:, kt, :]),
                    start=False, stop=(kt == hc - 1))
        func = AF.Tanh if g == 2 else AF.Sigmoid
        nc.scalar.activation(
            out=gb[g][:, jc % hc, :], in_=ps[:], func=func,
            bias=bias_sb[:, jc:jc + 1], scale=1.0)
    for s in range(hc):
        i_, g_, o_ = gb[0][:, s, :], gb[2][:, s, :], gb[3][:, s, :]
        if cprevT is None:
            nc.vector.tensor_mul(c_out[:, s, :], i_, g_)
        else:
            f_ = gb[1][:, s, :]
            ig = tp.tile([P, NL], F32, tag="ig", bufs=2, name="ig")
            nc.vector.tensor_mul(ig[:], i_, g_)
            nc.vector.tensor_mul(c_out[:, s, :], f_, cprevT[:, s, :])
            nc.vector.tensor_add(c_out[:, s, :], c_out[:, s, :], ig[:])
        tc_ = tp.tile([P, NL], F32, tag="tanhc", bufs=2, name="tanhc")
        nc.scalar.activation(out=tc_[:], in_=c_out[:, s, :], func=AF.Tanh)
        nc.vector.tensor_mul(h_out[:, s, :], o_, tc_[:])


def _gather_T(nc, pools, emb, idx_dram, ident, dstT):
    """Gather NL embedding rows and transpose into dstT (P, NE, NL)."""
    ip, rp, pt, cp = pools["idx"], pools["raw"], pools["pt"], pools["tmp"]
    for t in range(NL // P):
        idx_t = ip.tile([P, 1], I32, tag="idx", bufs=4, name="idx_t")
        nc.sync.dma_start(out=idx_t[:], in_=idx_dram[t * P:(t + 1) * P, :])
        raw = rp.tile([P, E], F32, tag="raw", bufs=4, name="raw")
        nc.gpsimd.indirect_dma_start(
            out=raw[:], out_offset=None, in_=emb[:],
            in_offset=bass.IndirectOffsetOnAxis(ap=idx_t[:, :1], axis=0))
        for et in range(NE):
            ptile = pt.tile([P, P], F32, tag="pt", bufs=4, name="ptile")
            nc.tensor.transpose(
                out=ptile[:], in_=raw[:, et * P:(et + 1) * P], identity=ident[:])
            nc.vector.tensor_copy(
                out=dstT[:, et, t * P:(t + 1) * P], in_=ptile[:])


def build_program():
    nc = bacc.Bacc("TRN2", target_bir_lowering=False, debug=False,
                   enable_asserts=False, num_devices=NCORES)
    dram = lambda name, shape, dt=F32, kind="ExternalInput": \
        nc.dram_tensor(name, shape, dt, kind=kind).ap()

    emb = dram("emb", [V, E])
    idx_x = dram("idx_x", [NL, 1], I32)
    idx_e0 = dram("idx_e0", [NL, 1], I32)
    idx_e1 = dram("idx_e1", [NL, 1], I32)
    # weights pre-laid-out on host as lhsT tiles [p, kt, j]
    wgf = dram("wgf", [P, NE, 4 * G], F32R)
    wgr = dram("wgr", [P, NE, 4 * G], F32R)
    ugf = dram("ugf", [P, NH, 4 * G], F32R)
    ugr = dram("ugr", [P, NH, 4 * G], F32R)
    wf = dram("wf", [P, NE, 4 * E], F32R)
    bgf = dram("bgf", [P, 8])
    bgr = dram("bgr", [P, 8])
    bf = dram("bf", [P, 16])
    out_dot = dram("out_dot", [2, E], kind="ExternalOutput")
    out_sr = dram("out_sr", [2, E], kind="ExternalOutput")
    out_sg = dram("out_sg", [2, E], kind="ExternalOutput")

    with tile.TileContext(nc) as tc:
        _emit(tc, locals())
    nc.compile()
    return nc


PHASE = int(os.environ.get("KBENCH_PHASE", "4"))


def _dummy_outs(nc, pool, T):
    z = pool.tile([P, 12], F32, name="zeros")
    nc.vector.memset(z[:], 0.0)
    for nm in ("out_dot", "out_sr", "out_sg"):
        for b in range(2):
            for et in range(NE):
                nc.sync.dma_start(out=T[nm][b, et * P:(et + 1) * P],
                                  in_=z[:, b * 6 + et:b * 6 + et + 1])


def _emit(tc, T):
    nc = tc.nc
    rg = [list(range(NCORES))]
    BF16 = mybir.dt.bfloat16
    from contextlib import ExitStack
    ctx = ExitStack()
    with ctx:
        glob = ctx.enter_context(tc.tile_pool(name="glob", bufs=1))
        dramp = ctx.enter_context(tc.tile_pool(name="dramp", bufs=1, space="DRAM"))

        ident = glob.tile([P, P], F32)
        make_identity(nc, ident)
        identr = glob.tile([P, P], F32R)
        nc.vector.tensor_copy(out=identr[:], in_=ident[:])

        # collective bounce buffers: fh/logits side fp32r, value side bf16
        ag1_src = dramp.tile([E, NL], F32R)                      # fh.T local
        ag1_dst = dramp.tile([NCORES * E, NL], F32R, addr_space="Shared")
        # declared with wide rows (fewer rows, same bytes) — collective cost
        # scales with descriptor rows; access via the narrow row view below
        ag2_src_w = dramp.tile([NL, 2 * E], BF16)
        ag2_dst_w = dramp.tile([NCORES * NL, 2 * E], BF16, addr_space="Shared")
        ag2_src = ag2_src_w.rearrange("a (r b) -> (a r) b", r=2)  # (2NL, E)
        ag2_dst = ag2_dst_w.rearrange("a (r b) -> (a r) b", r=2)

        # long-lived local activations
        g0T = glob.tile([P, NE, NL], F32R)
        g1T = glob.tile([P, NE, NL], F32R)
        dgT = glob.tile([P, NE, NL], F32R)

        with tc.tile_pool(name="wpool", bufs=1) as wp, \
             tc.tile_pool(name="acts", bufs=1) as ap_, \
             tc.tile_pool(name="gates", bufs=1) as gp, \
             tc.tile_pool(name="tmp", bufs=1) as tp, \
             tc.tile_pool(name="idx", bufs=1) as ip, \
             tc.tile_pool(name="raw", bufs=1) as rp, \
             tc.tile_pool(name="pg", bufs=1, space="PSUM") as pgp, \
             tc.tile_pool(name="pt", bufs=1, space="PSUM") as ptp:
            pools = {"pg": pgp, "gates": gp, "tmp": tp, "idx": ip,
                     "raw": rp, "pt": ptp}

            # -- e gathers first: the g-LSTM is the long pole to AG2 --
            e0T = ap_.tile([P, NE, NL], F32R)
            e1T = ap_.tile([P, NE, NL], F32R)
            _gather_T(nc, pools, T["emb"], T["idx_e0"], ident, e0T)
            _gather_T(nc, pools, T["emb"], T["idx_e1"], ident, e1T)
            w_sb = {}
            for nm, kt in (("wgf", NE), ("wgr", NE), ("ugf", NH), ("ugr", NH)):
                w_sb[nm] = wp.tile([P, kt, 4 * G], F32R, name=nm + "_sb")
                nc.sync.dma_start(out=w_sb[nm][:], in_=T[nm][:])
            for nm in ("bgf", "bgr"):
                w_sb[nm] = wp.tile([P, 8], F32, name=nm + "_sb")
                nc.sync.dma_start(out=w_sb[nm][:], in_=T[nm][:])

            # two independent chains: (fwd0 -> fwd1) and (rev1 -> rev0)
            cfT = ap_.tile([P, NH, NL], F32, name="cfT")
            crT = ap_.tile([P, NH, NL], F32, name="crT")
            c2T = ap_.tile([P, NH, NL], F32, name="c2T")
            c3T = ap_.tile([P, NH, NL], F32, name="c3T")
            hf0 = g0T[:, 0:NH, :]
            hf1 = g1T[:, 0:NH, :]
            hr1 = g1T[:, NH:NE, :]
            hr0 = g0T[:, NH:NE, :]
            _lstm_cell(nc, pools, G, e0T, w_sb["wgf"], None, None, None,
                       w_sb["bgf"], hf0, cfT)
            _lstm_cell(nc, pools, G, e1T, w_sb["wgr"], None, None, None,
                       w_sb["bgr"], hr1, crT)
            # -- x gather, f weights, f-LSTM, AG1 --
            xT = ap_.tile([P, NE, NL], F32R)
            _gather_T(nc, pools, T["emb"], T["idx_x"], ident, xT)
            wf_sb = wp.tile([P, NE, 4 * E], F32R)
            nc.sync.dma_start(out=wf_sb[:], in_=T["wf"][:])
            bf_sb = wp.tile([P, 16], F32)
            nc.sync.dma_start(out=bf_sb[:], in_=T["bf"][:])

            fhT = ap_.tile([P, NE, NL], F32R)
            _lstm_cell(nc, pools, E, xT, wf_sb, None, None, None, bf_sb, fhT,
                       None)
            for et in range(NE):
                nc.vector.tensor_add(fhT[:, et, :], fhT[:, et, :], xT[:, et, :])
                nc.sync.dma_start(
                    out=ag1_src[et * P:(et + 1) * P, :], in_=fhT[:, et, :])
            nc.gpsimd.collective_compute(
                "AllGather", ALU.bypass, replica_groups=rg,
                ins=[ag1_src[:].opt()], outs=[ag1_dst[:].opt()])
            _lstm_cell(nc, pools, G, e1T, w_sb["wgf"], w_sb["ugf"], hf0, cfT,
                       w_sb["bgf"], hf1, c2T)
            _lstm_cell(nc, pools, G, e0T, w_sb["wgr"], w_sb["ugr"], hr1, crT,
                       w_sb["bgr"], hr0, c3T)


            for et in range(NE):
                nc.vector.tensor_sub(dgT[:, et, :], g0T[:, et, :], g1T[:, et, :])

        # -- transpose g0/g1 into ag2_src (bf16) and fire AG2 early --
        NMB = N // P  # 32 m-blocks
        attn = ctx.enter_context(tc.tile_pool(name="attn", bufs=1))
        A0T = attn.tile([P, NMB, NL], mybir.dt.bfloat16)
        with tc.tile_pool(name="tps", bufs=1) as tsp, \
             tc.tile_pool(name="fin", bufs=1) as fin, \
             tc.tile_pool(name="fhk", bufs=1) as fkp, \
             tc.tile_pool(name="ptg", bufs=1, space="PSUM") as ptgp, \
             tc.tile_pool(name="pd", bufs=1, space="PSUM") as pdp:
            with tc.high_priority():
                for srcT, row0 in ((g0T, 0), (g1T, NL)):
                    for nt in range(NL // P):
                        ptile = ptgp.tile([P, E], F32R, tag="ptg", bufs=3,
                                          name="ptg")
                        for et in range(NE):
                            nc.tensor.transpose(
                                out=ptile[:, et * P:(et + 1) * P],
                                in_=srcT[:, et, nt * P:(nt + 1) * P],
                                identity=identr[:])
                        stile = tsp.tile([P, E], BF16, tag="tps", bufs=3,
                                         name="stile")
                        nc.vector.tensor_copy(out=stile[:],
                                              in_=ptile[:].bitcast(F32))
                        nc.sync.dma_start(
                            out=ag2_src[row0 + nt * P:row0 + (nt + 1) * P, :],
                            in_=stile[:])
                nc.gpsimd.collective_compute(
                    "AllGather", ALU.bypass, replica_groups=rg,
                    ins=[ag2_src_w[:].opt()], outs=[ag2_dst_w[:].opt()])

            # -- early reductions: sg_b = sum_n g_b^2 (local) --
            for b, gT in ((0, g0T), (1, g1T)):
                for et in range(NE):
                    scr3 = fin.tile([P, NL], F32, tag="scr3", bufs=2,
                                    name="scr3")
                    asg = fin.tile([P, 1], F32, tag="asg", bufs=2, name="asg")
                    nc.scalar.activation(out=scr3[:],
                                         in_=gT[:, et, :].bitcast(F32),
                                         func=AF.Square, accum_out=asg[:])
                    nc.sync.dma_start(out=T["out_sg"][b, et * P:(et + 1) * P],
                                      in_=asg[:])

            # -- phase D1: D.T = fh.T-blocks x dgT; A0 = sigmoid(D) (bf16) --
            for k in range(NCORES):
                fhk = fkp.tile([P, NE, NL], F32R, tag="fhk", bufs=3, name="fhk")
                nc.sync.dma_start(
                    out=fhk[:],
                    in_=ag1_dst[k * E:(k + 1) * E, :].rearrange(
                        "(et p) n -> p et n", p=P))
                for c in range(NL // P):
                    mb = k * (NL // P) + c
                    pd = pdp.tile([P, NL], F32, tag="pd", bufs=3, name="pd")
                    cs = slice(c * P, (c + 1) * P)
                    for et in range(NE):
                        nc.tensor.matmul(
                            pd[:], fhk[:, et, cs], dgT[:, et, :],
                            start=(et == 0), stop=(et == NE - 1))
                    nc.scalar.activation(
                        out=A0T[:, mb, :], in_=pd[:], func=AF.Sigmoid)

        # ---- phase D2: r0.T / r1.T accumulate over all m-blocks in PSUM ----
        with tc.tile_pool(name="gb", bufs=1) as gbp, \
             tc.tile_pool(name="pr", bufs=1, space="PSUM") as prp, \
             tc.tile_pool(name="fin2", bufs=1) as fin:
            BF = mybir.dt.bfloat16
            r0p = [prp.tile([P, NL], F32, tag=f"r0_{et}", name=f"r0_{et}")
                   for et in range(NE)]
            r1p = [prp.tile([P, NL], F32, tag=f"r1_{et}", name=f"r1_{et}")
                   for et in range(NE)]
            for mb in range(NMB):
                k, c = divmod(mb, NL // P)
                base = k * 2 * NL
                g0b = gbp.tile([P, E], BF, tag="g0b", bufs=4, name="g0b")
                nc.sync.dma_start(
                    out=g0b[:], in_=ag2_dst[base + c * P:base + (c + 1) * P, :])
                g1b = gbp.tile([P, E], BF, tag="g1b", bufs=4, name="g1b")
                nc.sync.dma_start(
                    out=g1b[:],
                    in_=ag2_dst[base + NL + c * P:base + NL + (c + 1) * P, :])
                a1 = gbp.tile([P, NL], BF, tag="a1", bufs=4, name="a1")
                nc.vector.tensor_scalar(
                    out=a1[:], in0=A0T[:, mb, :], scalar1=-1.0, scalar2=1.0,
                    op0=ALU.mult, op1=ALU.add)
                for et in range(NE):
                    es = slice(et * P, (et + 1) * P)
                    nc.tensor.matmul(
                        r0p[et][:], g0b[:, es], A0T[:, mb, :],
                        start=(mb == 0), stop=(mb == NMB - 1))
                    nc.tensor.matmul(
                        r1p[et][:], g1b[:, es], a1[:],
                        start=(mb == 0), stop=(mb == NMB - 1))

            # ---- phase E: dot and r^2 reductions over local n ----
            for b, (rp_, gT) in enumerate(((r0p, g0T), (r1p, g1T))):
                for et in range(NE):
                    scr2 = fin.tile([P, NL], F32, tag="scr2", bufs=4,
                                    name="scr2")
                    asr = fin.tile([P, 1], F32, tag="asr", bufs=4, name="asr")
                    nc.scalar.activation(out=scr2[:], in_=rp_[et][:],
                                         func=AF.Square, accum_out=asr[:])
                    nc.sync.dma_start(out=T["out_sr"][b, et * P:(et + 1) * P],
                                      in_=asr[:])
                scr = fin.tile([P, NE, NL], F32, tag="scr", bufs=2, name="scr")
                adot = fin.tile([P, NE, 1], F32, tag="adot", bufs=2,
                                name="adot")
                for et in range(NE):
                    nc.vector.tensor_mul(scr[:, et, :], rp_[et][:],
                                         gT[:, et, :].bitcast(F32))
                nc.vector.reduce_sum(out=adot[:], in_=scr[:],
                                     axis=mybir.AxisListType.X)
                for et in range(NE):
                    nc.sync.dma_start(out=T["out_dot"][b, et * P:(et + 1) * P],
                                      in_=adot[:, et, :])



_PROGRAM = None


def _get_program():
    global _PROGRAM
    if _PROGRAM is None:
        _PROGRAM = build_program()
    return _PROGRAM


def _prep_w(w):
    """(4H, E_in) torch-layout weight -> lhsT tiles [p, kt, 4H]."""
    wt = np.asarray(w, np.float32).T  # (E_in, 4H)
    e_in, fourh = wt.shape
    return np.ascontiguousarray(wt.reshape(e_in // P, P, fourh).transpose(1, 0, 2))


def _prep_b(b1, b2):
    s = (np.asarray(b1, np.float32) + np.asarray(b2, np.float32))
    return np.ascontiguousarray(s.reshape(-1, P).T)


def run_device(inputs, trace=False):
    """Shard inputs, run the 8-core SPMD program, return (results, bass results)."""
    nc = _get_program()
    emb = np.ascontiguousarray(np.asarray(inputs["embedding"], np.float32))
    iq = np.asarray(inputs["input"]).astype(np.int32).reshape(N, 1)
    ie = np.asarray(inputs["set_inputs"]).astype(np.int32)
    shared = {
        "emb": emb,
        "wgf": _prep_w(inputs["wih_gf"]), "wgr": _prep_w(inputs["wih_gr"]),
        "ugf": _prep_w(inputs["whh_gf"]), "ugr": _prep_w(inputs["whh_gr"]),
        "wf": _prep_w(inputs["wih_f"]),
        "bgf": _prep_b(inputs["bih_gf"], inputs["bhh_gf"]),
        "bgr": _prep_b(inputs["bih_gr"], inputs["bhh_gr"]),
        "bf": _prep_b(inputs["bih_f"], inputs["bhh_f"]),
    }
    in_maps = []
    for k in range(NCORES):
        sl = slice(k * NL, (k + 1) * NL)
        m = dict(shared)
        m["idx_x"] = np.ascontiguousarray(iq[sl])
        m["idx_e0"] = np.ascontiguousarray(ie[0, sl].reshape(NL, 1))
        m["idx_e1"] = np.ascontiguousarray(ie[1, sl].reshape(NL, 1))
        in_maps.append(m)
    res = bass_utils.run_bass_kernel_spmd(
        nc, in_maps, core_ids=list(range(NCORES)), trace=trace)
    return res


def kernel(**inputs):
    res = run_device(inputs)
    return host_tail(res, inputs)


def host_tail(res, inputs):
    dot = np.zeros((2, E), np.float64)
    sr = np.zeros((2, E), np.float64)
    sg = np.zeros((2, E), np.float64)
    for r in res.results:
        dot += r["out_dot"]
        sr += r["out_sr"]
        sg += r["out_sg"]
    nr = np.maximum(np.sqrt(sr), EPS)
    ng = np.maximum(np.sqrt(sg), EPS)
    cos = dot / (nr * ng)                        # (2, E)
    kern = cos / np.exp(cos).sum()
    w_out = np.asarray(inputs["w_out"], np.float64)
    b_out = np.asarray(inputs["b_out"], np.float64)
    k2 = kern @ w_out.T + b_out                  # (2, R)
    s = k2.sum(axis=1)                           # (2,)
    labels = np.asarray(inputs["set_labels"], np.float64)
    o = s[0] * labels[0] + s[1] * labels[1]      # (R,)
    o = np.exp(o - o.max())
    o /= o.sum()
    return o.astype(np.float32)



# revision 21
# speedup vs baseline: 1.4440x; 1.4440x over previous
"""MatchingNet model kernel for 8 Trainium2 NeuronCores — fp8 v3.

Reference semantics (N=4096, E=512, G=256, V=50000, R=1000):
  x  = embedding[input]          (N, E)
  ex = embedding[set_inputs]     (2, N, E)
  g_out = bidirectional 2-step LSTM over ex   (2, N, E)
  fh = lstm_f(x) + x             (N, E)
  scores[b] = g_out[b] @ fh.T ; a = softmax(scores, axis=0)
  r[b] = a[b] @ g_out[b] ; cosine-reduce over n -> tiny host tail

Sharding: data-parallel over N; core k owns rows [512k, 512k+512).
Everything runs in fp8e4m3 (DoubleRow matmuls, fp8 collectives, fp8 g
storage); a numpy simulation of this dataflow has rel err ~3e-5 against
the fp32 reference (gate is 2e-2).

v3 structure (from the v2 trace): all input-side gate matmuls of all
five LSTM cells are issued as one continuous PE stream (keeps the PE
out of low p-state); the recurrent cells' x-parts spill to bf16 and are
re-added after the U @ h matmul; h tensors are written directly as fp8
(no big casts); the c-state elementwise chain runs on Pool, h-muls and
adds on DVE, activations on Act — three engines pipelined.
a0 = sigmoid((g0-g1) @ fh.T); r1 is never formed: q1 = a0 @ g1 and the
b=1 reductions collapse algebraically on the host via S1 = colsum(g1).
All 8 per-core reduction rows collect in one staging tile, transposed
and written with a single DMA at the end.
"""

import os
import sys

import numpy as np

for _p in ("/opt/trn_rl_repo", os.path.expanduser("~/.axon_site/_ro/trn_rl_repo")):
    if os.path.isdir(_p) and _p not in sys.path:
        sys.path.insert(0, _p)

import concourse.bacc as bacc
import concourse.bass as bass
import concourse.mybir as mybir
import concourse.tile as tile
from concourse import bass_utils
from concourse.masks import make_identity

N, E, G, V, R = 4096, 512, 256, 50000, 1000
NCORES = 8
NL = N // NCORES  # 512 rows per core
P = 128
NE = E // P   # 4 e-chunks
NH = G // P   # 2 hidden chunks for the g-LSTM
NMB = N // P  # 32 m-blocks
EPS = 1e-8

F32 = mybir.dt.float32
BF16 = mybir.dt.bfloat16
F8 = mybir.dt.float8e4
I32 = mybir.dt.int32
AF = mybir.ActivationFunctionType
ALU = mybir.AluOpType
DR = mybir.MatmulPerfMode.DoubleRow


def _gather8(nc, pools, emb8, idx_dram, ident8, dstT8):
    """Gather NL fp8 embedding rows and transpose into dstT8 (P, NE, NL)."""
    ip, rp, pt = pools["idx"], pools["raw"], pools["pt"]
    for t in range(NL // P):
        idx_t = ip.tile([P, 1], I32, tag="idx", bufs=4, name="idx_t")
        nc.sync.dma_start(out=idx_t[:], in_=idx_dram[t * P:(t + 1) * P, :])
        raw = rp.tile([P, E], F8, tag="raw", bufs=4, name="raw")
        nc.gpsimd.indirect_dma_start(
            out=raw[:], out_offset=None, in_=emb8[:],
            in_offset=bass.IndirectOffsetOnAxis(ap=idx_t[:, :1], axis=0))
        # fp8 transpose outputs must land with element step 2 (HW rule)
        ptile = pt.tile([P, NE, P, 2], F8, tag="pt", bufs=2, name="ptile")
        for et in range(NE):
            nc.tensor.transpose(
                out=ptile[:, et, :, 0], in_=raw[:, et * P:(et + 1) * P],
                identity=ident8[:])
        nc.vector.tensor_copy(
            out=dstT8[:, :, t * P:(t + 1) * P], in_=ptile[:, :, :, 0])


def _xgates(nc, pg, xT8, W_sb, b8_sb, ones8, gates, hc, s0=0):
    """Stream the x-side gate matmuls for (gate, chunk-pair) into PSUM.

    Returns {gate: psum tile [P, 2, NL]} covering feature chunks
    (g*hc + s0, g*hc + s0 + 1). Bias is a K=1 matmul against a ones row.
    """
    ps = {}
    for g in gates:
        t = pg.tile([P, 2, NL], F32, tag="pg2", bufs=3, name="ps_gate")
        for s in range(2):
            jc = g * hc + s0 + s
            js = slice(jc * P, (jc + 1) * P)
            nc.tensor.matmul(t[:, s, :], b8_sb[:, js], ones8[:],
                             start=True, stop=False, skip_group_check=True)
            for i in range(NE // 2):
                nc.tensor.matmul(
                    t[:, s, :], W_sb[:, 2 * i:2 * i + 2, js],
                    xT8[:, 2 * i:2 * i + 2, :],
                    start=False, stop=(i == NE // 2 - 1),
                    perf_mode=DR, skip_group_check=True)
        ps[g] = t
    return ps


def build_program():
    nc = bacc.Bacc("TRN2", target_bir_lowering=False, debug=False,
                   enable_asserts=False, num_devices=NCORES)
    dram = lambda name, shape, dt=F32, kind="ExternalInput": \
        nc.dram_tensor(name, shape, dt, kind=kind).ap()

    emb8 = dram("emb8", [V, E], F8)
    idx_x = dram("idx_x", [NL, 1], I32)
    idx_e0 = dram("idx_e0", [NL, 1], I32)
    idx_e1 = dram("idx_e1", [NL, 1], I32)
    wgf = dram("wgf", [P, NE, 4 * G], F8)
    wgr = dram("wgr", [P, NE, 4 * G], F8)
    ugf = dram("ugf", [P, NH, 4 * G], F8)
    ugr = dram("ugr", [P, NH, 4 * G], F8)
    wf = dram("wf", [P, NE, 4 * E], F8)
    bgf = dram("bgf", [1, 4 * G], F8)
    bgr = dram("bgr", [1, 4 * G], F8)
    bf = dram("bf", [1, 4 * E], F8)
    out = dram("out", [8, E], kind="ExternalOutput")

    with tile.TileContext(nc) as tc:
        _emit(tc, locals())
    nc.compile()
    return nc


def _emit(tc, T):
    nc = tc.nc
    rg = [list(range(NCORES))]
    from contextlib import ExitStack
    ctx = ExitStack()
    with ctx:
        glob = ctx.enter_context(tc.tile_pool(name="glob", bufs=1))
        dramp = ctx.enter_context(tc.tile_pool(name="dramp", bufs=1,
                                               space="DRAM"))

        identf = glob.tile([P, P], F32)
        make_identity(nc, identf)
        ident8 = glob.tile([P, P], F8)
        nc.vector.tensor_copy(out=ident8[:], in_=identf[:])
        ones8 = glob.tile([1, NL], F8)
        nc.vector.memset(ones8[:], 1.0)
        # staging for the 8 per-core reduction rows; one DMA at the end
        stag = glob.tile([P, 8, NE], F32)

        # collective bounce buffers (declared wide: fewer descriptor rows)
        ag1_src_w = dramp.tile([P, 4 * NL], F8)
        ag1_dst_w = dramp.tile([NCORES * P, 4 * NL], F8, addr_space="Shared")
        ag1s = ag1_src_w.rearrange("a (r b) -> (a r) b", r=4)    # (E, NL)
        ag1d = ag1_dst_w.rearrange("a (r b) -> (a r) b", r=4)    # (8E, NL)
        ag2_src_w = dramp.tile([2 * NL // 4, 4 * E], F8)
        ag2_dst_w = dramp.tile([NCORES * 2 * NL // 4, 4 * E], F8,
                               addr_space="Shared")
        ag2s = ag2_src_w.rearrange("a (r b) -> (a r) b", r=4)    # (2NL, E)
        ag2d = ag2_dst_w.rearrange("a (r b) -> (a r) b", r=4)    # (8*2NL, E)

        # fp8 activations (g_out lives fp8 end-to-end)
        g08 = glob.tile([P, NE, NL], F8)   # [hf0 | hr0]
        g18 = glob.tile([P, NE, NL], F8)   # [hf1 | hr1]
        dgT8 = glob.tile([P, NE, NL], F8)
        A0T = glob.tile([P, NMB, NL], F8)

        with tc.tile_pool(name="wpool", bufs=1) as wp, \
             tc.tile_pool(name="acts", bufs=1) as ap_, \
             tc.tile_pool(name="gates", bufs=1) as gp, \
             tc.tile_pool(name="tmp", bufs=1) as tp, \
             tc.tile_pool(name="idx", bufs=1) as ip, \
             tc.tile_pool(name="raw", bufs=1) as rp, \
             tc.tile_pool(name="pg", bufs=1, space="PSUM") as pgp, \
             tc.tile_pool(name="pt", bufs=1, space="PSUM") as ptp:
            pools = {"idx": ip, "raw": rp, "pt": ptp}

            # ---- gathers + weight loads ----
            xT8 = ap_.tile([P, NE, NL], F8)
            e0T8 = ap_.tile([P, NE, NL], F8)
            e1T8 = ap_.tile([P, NE, NL], F8)
            _gather8(nc, pools, T["emb8"], T["idx_x"], ident8, xT8)
            _gather8(nc, pools, T["emb8"], T["idx_e0"], ident8, e0T8)
            _gather8(nc, pools, T["emb8"], T["idx_e1"], ident8, e1T8)
            w_sb = {}
            for nm, kt in (("wgf", NE), ("wgr", NE), ("wf", NE),
                           ("ugf", NH), ("ugr", NH)):
                hw = 4 * (E if nm == "wf" else G)
                w_sb[nm] = wp.tile([P, kt, hw], F8, name=nm + "_sb")
                nc.sync.dma_start(out=w_sb[nm][:], in_=T[nm][:])
            for nm, hw in (("bgf", 4 * G), ("bgr", 4 * G), ("bf", 4 * E)):
                w_sb[nm] = wp.tile([1, hw], F8, name=nm + "_sb")
                nc.sync.dma_start(out=w_sb[nm][:], in_=T[nm][:])

            # ---- one continuous PE stream: all x-side gate matmuls ----
            ZG = (0, 2, 3)  # i, g, o (forget unused with zero state)
            c1ps = _xgates(nc, pgp, e0T8, w_sb["wgf"], w_sb["bgf"], ones8,
                           ZG, NH)
            c2ps = _xgates(nc, pgp, e1T8, w_sb["wgr"], w_sb["bgr"], ones8,
                           ZG, NH)
            fps = [_xgates(nc, pgp, xT8, w_sb["wf"], w_sb["bf"], ones8,
                           ZG, NE, s0=2 * h) for h in range(2)]
            c3ps = _xgates(nc, pgp, e1T8, w_sb["wgf"], w_sb["bgf"], ones8,
                           (0, 1, 2, 3), NH)
            c4ps = _xgates(nc, pgp, e0T8, w_sb["wgr"], w_sb["bgr"], ones8,
                           (0, 1, 2, 3), NH)
            # spill recurrent cells' x-parts to bf16 (psum is scarce)
            xp3 = ap_.tile([P, 4, 2, NL], BF16, name="xp3")
            xp4 = ap_.tile([P, 4, 2, NL], BF16, name="xp4")
            for g in range(4):
                eng = nc.vector if g % 2 else nc.scalar
                if eng is nc.scalar:
                    nc.scalar.copy(out=xp3[:, g], in_=c3ps[g][:])
                    nc.scalar.copy(out=xp4[:, g], in_=c4ps[g][:])
                else:
                    nc.vector.tensor_copy(out=xp3[:, g], in_=c3ps[g][:])
                    nc.vector.tensor_copy(out=xp4[:, g], in_=c4ps[g][:])

            def zcell(ps, h_out, c_out):
                """Zero-state cell tail: acts + c/h elementwise."""
                gi = gp.tile([P, 2, NL], F32, tag="gi", bufs=2, name="gi")
                gg = gp.tile([P, 2, NL], F32, tag="gg", bufs=2, name="gg")
                go = gp.tile([P, 2, NL], F32, tag="go", bufs=2, name="go")
                nc.scalar.activation(out=gi[:], in_=ps[0][:], func=AF.Sigmoid)
                nc.scalar.activation(out=gg[:], in_=ps[2][:], func=AF.Tanh)
                nc.scalar.activation(out=go[:], in_=ps[3][:], func=AF.Sigmoid)
                nc.gpsimd.tensor_mul(c_out[:], gi[:], gg[:])
                tc_ = tp.tile([P, 2, NL], F32, tag="tanhc", bufs=2,
                              name="tanhc")
                nc.scalar.activation(out=tc_[:], in_=c_out[:], func=AF.Tanh)
                nc.vector.tensor_mul(h_out[:], go[:], tc_[:])

            cfT = ap_.tile([P, NH, NL], F32, name="cfT")
            crT = ap_.tile([P, NH, NL], F32, name="crT")
            zcell(c1ps, g08[:, 0:NH, :], cfT)    # hf0
            zcell(c2ps, g18[:, NH:NE, :], crT)   # hr1

            # f-cell (two feature halves) -> fh8 = h + x, fire AG1
            fh8 = ap_.tile([P, NE, NL], F8, name="fh8")
            with tc.high_priority():
                for h in range(2):
                    hs = slice(2 * h, 2 * h + 2)
                    cf_ = tp.tile([P, 2, NL], F32, tag="cf", bufs=2,
                                  name="cf")
                    hf_ = tp.tile([P, 2, NL], F32, tag="hf", bufs=2,
                                  name="hf")
                    zcell(fps[h], hf_, cf_)
                    nc.vector.tensor_add(fh8[:, hs, :], hf_[:], xT8[:, hs, :])
                nc.sync.dma_start(
                    out=ag1s[:].rearrange("(et p) n -> p et n", p=P),
                    in_=fh8[:])
                nc.gpsimd.collective_compute(
                    "AllGather", ALU.bypass, replica_groups=rg,
                    ins=[ag1_src_w[:].opt()], outs=[ag1_dst_w[:].opt()])

            def rcell(xp, U_sb, hprev8, cprev, h_out):
                """Recurrent cell tail: U@h + spilled x-part, acts, c/h."""
                gb = []
                for g in range(4):
                    ups = pgp.tile([P, 2, NL], F32, tag="pg2", bufs=3,
                                   name="ups")
                    for s in range(2):
                        nc.tensor.matmul(
                            ups[:, s, :], U_sb[:, :, (g * NH + s) * P:
                                                (g * NH + s + 1) * P],
                            hprev8[:], start=True, stop=True, perf_mode=DR,
                            skip_group_check=True)
                    gt = gp.tile([P, 2, NL], F32, tag=f"rg{g}", bufs=2,
                                 name=f"rg{g}")
                    nc.vector.tensor_add(gt[:], ups[:], xp[:, g])
                    func = AF.Tanh if g == 2 else AF.Sigmoid
                    nc.scalar.activation(out=gt[:], in_=gt[:], func=func)
                    gb.append(gt)
                ig = tp.tile([P, 2, NL], F32, tag="ig", bufs=2, name="ig")
                nc.gpsimd.tensor_mul(ig[:], gb[0][:], gb[2][:])
                cc = tp.tile([P, 2, NL], F32, tag="cc", bufs=2, name="cc")
                nc.gpsimd.tensor_mul(cc[:], gb[1][:], cprev[:])
                nc.gpsimd.tensor_add(cc[:], cc[:], ig[:])
                tc_ = tp.tile([P, 2, NL], F32, tag="tanhc", bufs=2,
                              name="tanhc")
                nc.scalar.activation(out=tc_[:], in_=cc[:], func=AF.Tanh)
                nc.vector.tensor_mul(h_out[:], gb[3][:], tc_[:])

            rcell(xp3, w_sb["ugf"], g08[:, 0:NH, :], cfT,
                  g18[:, 0:NH, :])   # hf1
            rcell(xp4, w_sb["ugr"], g18[:, NH:NE, :], crT,
                  g08[:, NH:NE, :])  # hr0

            nc.vector.tensor_sub(dgT8[:], g08[:], g18[:])

            # transpose g0/g1 to n-major and fire AG2
            with tc.high_priority():
                for src8, row0 in ((g08, 0), (g18, NL)):
                    for nt in range(NL // P):
                        ptile = ptp.tile([P, NE, P, 2], F8, tag="pt", bufs=2,
                                         name="ptg")
                        for et in range(NE):
                            nc.tensor.transpose(
                                out=ptile[:, et, :, 0],
                                in_=src8[:, et, nt * P:(nt + 1) * P],
                                identity=ident8[:])
                        stile = tp.tile([P, E], F8, tag="tps", bufs=3,
                                        name="stile")
                        nc.vector.tensor_copy(
                            out=stile[:].rearrange("p (et q) -> p et q", q=P),
                            in_=ptile[:, :, :, 0])
                        nc.sync.dma_start(
                            out=ag2s[row0 + nt * P:row0 + (nt + 1) * P, :],
                            in_=stile[:])
                nc.gpsimd.collective_compute(
                    "AllGather", ALU.bypass, replica_groups=rg,
                    ins=[ag2_src_w[:].opt()], outs=[ag2_dst_w[:].opt()])

            # early local reductions -> staging rows 2 (sg0), 3 (sg1),
            # 7 (S1c = local colsum of g1)
            for row, gT in ((2, g08), (3, g18)):
                for et in range(NE):
                    scr = tp.tile([P, NL], F32, tag="scr0", bufs=2,
                                  name="scr0")
                    nc.vector.tensor_mul(scr[:], gT[:, et, :], gT[:, et, :])
                    nc.vector.reduce_sum(out=stag[:, row, et:et + 1],
                                         in_=scr[:],
                                         axis=mybir.AxisListType.X)
            for et in range(NE):
                nc.vector.reduce_sum(out=stag[:, 7, et:et + 1],
                                     in_=g18[:, et, :],
                                     axis=mybir.AxisListType.X)

        # ---- phase C: preloads + D1 = sigmoid((g0-g1) @ fh^T) ----
        dp = ctx.enter_context(tc.tile_pool(name="dpool", bufs=1))
        fhAll = dp.tile([P, NCORES, NE, NL], F8)
        for k in range(NCORES):
            nc.sync.dma_start(
                out=fhAll[:, k, :, :],
                in_=ag1d[k * E:(k + 1) * E, :].rearrange(
                    "(et p) n -> p et n", p=P))
        gAll = dp.tile([P, 2 * NMB, E], F8)
        for k in range(NCORES):
            nc.sync.dma_start(
                out=gAll[:, 4 * k:4 * k + 4, :],
                in_=ag2d[k * 2 * NL:k * 2 * NL + NL, :].rearrange(
                    "(c p) e -> p c e", p=P))
            nc.sync.dma_start(
                out=gAll[:, NMB + 4 * k:NMB + 4 * k + 4, :],
                in_=ag2d[k * 2 * NL + NL:(k + 1) * 2 * NL, :].rearrange(
                    "(c p) e -> p c e", p=P))

        with tc.tile_pool(name="pd", bufs=1, space="PSUM") as pdp:
            for k in range(NCORES):
                for cp in range(2):
                    pd2 = pdp.tile([P, 2, NL], F32, tag="pd", bufs=3,
                                   name="pd2")
                    for cc in range(2):
                        c = 2 * cp + cc
                        for i in range(NE // 2):
                            nc.tensor.matmul(
                                pd2[:, cc, :],
                                fhAll[:, k, 2 * i:2 * i + 2,
                                      c * P:(c + 1) * P],
                                dgT8[:, 2 * i:2 * i + 2, :],
                                start=(i == 0), stop=(i == NE // 2 - 1),
                                perf_mode=DR)
                    mb = 4 * k + 2 * cp
                    nc.scalar.activation(
                        out=A0T[:, mb:mb + 2, :], in_=pd2[:], func=AF.Sigmoid)

        # ---- phase D: r0 = a0@g0, q1 = a0@g1 (PSUM accum over m) ----
        with tc.tile_pool(name="pr", bufs=1, space="PSUM") as prp, \
             tc.tile_pool(name="fin", bufs=1) as fin:
            r0p = [prp.tile([P, NL], F32, tag=f"r0_{et}", name=f"r0_{et}")
                   for et in range(NE)]
            q1p = [prp.tile([P, NL], F32, tag=f"q1_{et}", name=f"q1_{et}")
                   for et in range(NE)]
            for t in range(NMB // 2):
                a0sl = A0T[:, 2 * t:2 * t + 2, :]
                for et in range(NE):
                    es = slice(et * P, (et + 1) * P)
                    nc.tensor.matmul(
                        r0p[et][:], gAll[:, 2 * t:2 * t + 2, es], a0sl,
                        start=(t == 0), stop=(t == NMB // 2 - 1),
                        perf_mode=DR)
                    nc.tensor.matmul(
                        q1p[et][:], gAll[:, NMB + 2 * t:NMB + 2 * t + 2, es],
                        a0sl,
                        start=(t == 0), stop=(t == NMB // 2 - 1),
                        perf_mode=DR)

            # ---- phase E: reductions over local n into staging ----
            # rows: 0=dot0 1=sr0 2=sg0 3=sg1 4=A(sum q1) 5=B(sum q1^2)
            #       6=C(sum q1 g1) 7=S1c(sum g1)
            for et in range(NE):
                scr2 = fin.tile([P, NL], F32, tag="scr2", bufs=2, name="scr2")
                nc.vector.tensor_mul(scr2[:], r0p[et][:], g08[:, et, :])
                nc.vector.reduce_sum(out=stag[:, 0, et:et + 1], in_=scr2[:],
                                     axis=mybir.AxisListType.X)
                scr3 = fin.tile([P, NL], F32, tag="scr2", bufs=2, name="scr3")
                nc.vector.tensor_mul(scr3[:], q1p[et][:], g18[:, et, :])
                nc.vector.reduce_sum(out=stag[:, 6, et:et + 1], in_=scr3[:],
                                     axis=mybir.AxisListType.X)
                junk = fin.tile([P, NL], F32, tag="junk", bufs=4, name="junk")
                nc.scalar.activation(out=junk[:], in_=r0p[et][:],
                                     func=AF.Square,
                                     accum_out=stag[:, 1, et:et + 1])
                junk2 = fin.tile([P, NL], F32, tag="junk", bufs=4,
                                 name="junk2")
                nc.scalar.activation(out=junk2[:], in_=q1p[et][:],
                                     func=AF.Square,
                                     accum_out=stag[:, 5, et:et + 1])
                junk3 = fin.tile([P, NL], F32, tag="junk", bufs=4,
                                 name="junk3")
                nc.scalar.activation(out=junk3[:], in_=q1p[et][:],
                                     func=AF.Identity,
                                     accum_out=stag[:, 4, et:et + 1])

        # single transposed output DMA: stag [P, 8, 4] -> out [8, E]
        with tc.tile_pool(name="po", bufs=1, space="PSUM") as pop, \
             tc.tile_pool(name="fo", bufs=1) as fop:
            ot = pop.tile([32, P], F32)
            nc.tensor.transpose(out=ot[:],
                                in_=stag[:].rearrange("p r e -> p (r e)"),
                                identity=identf[:])
            os_ = fop.tile([32, P], F32)
            nc.vector.tensor_copy(out=os_[:], in_=ot[:])
            nc.sync.dma_start(
                out=T["out"][:].rearrange("r (et p) -> (r et) p", p=P),
                in_=os_[:])


_PROGRAM = None


def _get_program():
    global _PROGRAM
    if _PROGRAM is None:
        _PROGRAM = build_program()
    return _PROGRAM


def _f8np():
    return mybir.dt.np(F8)


def _prep_w(w):
    """(4H, E_in) torch-layout weight -> fp8 lhsT tiles [p, kt, 4H]."""
    wt = np.asarray(w, np.float32).T  # (E_in, 4H)
    e_in, fourh = wt.shape
    t = np.ascontiguousarray(
        wt.reshape(e_in // P, P, fourh).transpose(1, 0, 2))
    return t.astype(_f8np())


def _prep_b(b1, b2):
    s = np.asarray(b1, np.float32) + np.asarray(b2, np.float32)
    return np.ascontiguousarray(s.reshape(1, -1)).astype(_f8np())


def run_device(inputs, trace=False):
    """Shard inputs, run the 8-core SPMD program, return results."""
    nc = _get_program()
    emb8 = np.asarray(inputs["embedding"], np.float32).astype(_f8np())
    iq = np.asarray(inputs["input"]).astype(np.int32).reshape(N, 1)
    ie = np.asarray(inputs["set_inputs"]).astype(np.int32)
    shared = {
        "emb8": np.ascontiguousarray(emb8),
        "wgf": _prep_w(inputs["wih_gf"]), "wgr": _prep_w(inputs["wih_gr"]),
        "ugf": _prep_w(inputs["whh_gf"]), "ugr": _prep_w(inputs["whh_gr"]),
        "wf": _prep_w(inputs["wih_f"]),
        "bgf": _prep_b(inputs["bih_gf"], inputs["bhh_gf"]),
        "bgr": _prep_b(inputs["bih_gr"], inputs["bhh_gr"]),
        "bf": _prep_b(inputs["bih_f"], inputs["bhh_f"]),
    }
    in_maps = []
    for k in range(NCORES):
        sl = slice(k * NL, (k + 1) * NL)
        m = dict(shared)
        m["idx_x"] = np.ascontiguousarray(iq[sl])
        m["idx_e0"] = np.ascontiguousarray(ie[0, sl].reshape(NL, 1))
        m["idx_e1"] = np.ascontiguousarray(ie[1, sl].reshape(NL, 1))
        in_maps.append(m)
    res = bass_utils.run_bass_kernel_spmd(
        nc, in_maps, core_ids=list(range(NCORES)), trace=trace)
    return res


def kernel(**inputs):
    res = run_device(inputs)
    return host_tail(res, inputs)


def host_tail(res, inputs):
    acc = np.zeros((8, E), np.float64)
    for r in res.results:
        acc += r["out"]
    dot0, sr0, sg0, sg1, A, B, C, S1 = acc
    dot1 = S1 * S1 - C
    sr1 = N * S1 * S1 - 2.0 * S1 * A + B
    dot = np.stack([dot0, dot1])
    sr = np.stack([sr0, sr1])
    sg = np.stack([sg0, sg1])
    nr = np.maximum(np.sqrt(sr), EPS)
    ng = np.maximum(np.sqrt(sg), EPS)
    cos = dot / (nr * ng)
    kern = cos / np.exp(cos).sum()
    w_out = np.asarray(inputs["w_out"], np.float64)
    b_out = np.asarray(inputs["b_out"], np.float64)
    k2 = kern @ w_out.T + b_out                  # (2, R)
    s = k2.sum(axis=1)                           # (2,)
    labels = np.asarray(inputs["set_labels"], np.float64)
    o = s[0] * labels[0] + s[1] * labels[1]      # (R,)
    o = np.exp(o - o.max())
    o /= o.sum()
    return o.astype(np.float32)


# revision 23
# speedup vs baseline: 1.7200x; 1.1911x over previous
"""MatchingNet model kernel for 8 Trainium2 NeuronCores — fp8 v4.

Reference semantics (N=4096, E=512, G=256, V=50000, R=1000):
  x  = embedding[input]          (N, E)
  ex = embedding[set_inputs]     (2, N, E)
  g_out = bidirectional 2-step LSTM over ex   (2, N, E)
  fh = lstm_f(x) + x             (N, E)
  scores[b] = g_out[b] @ fh.T ; a = softmax(scores, axis=0)
  r[b] = a[b] @ g_out[b] ; cosine-reduce over n -> tiny host tail

Sharding: data-parallel over N; core k owns rows [512k, 512k+512).
Everything runs in fp8e4m3 (DoubleRow matmuls contracting 256/instr,
fp8 collectives, fp8 g storage); numpy simulation of this dataflow has
rel err ~3e-5 against the fp32 reference (gate is 2e-2).

Attention: a0 = sigmoid((g0-g1) @ fh.T); r1 is never formed — q1 =
a0 @ g1 and the b=1 reductions collapse algebraically on the host via
S1 = colsum(g1):
  dot1 = S1^2 - sum_n q1*g1,  sr1 = N*S1^2 - 2*S1*sum q1 + sum q1^2.

Schedule notes (from v2/v3 traces): all x-side gate matmuls of all five
LSTM cells issue as one PE stream ordered so consecutive matmuls share
the moving tensor; gate biases ride the activation (per-chunk acts);
recurrent cells' x-parts spill to bf16, re-added after U @ h; c-state
chain on Pool, h-muls/adds on DVE, acts on Act — pipelined; the
all-gather preload DMAs ride the Act hwdge queue so they don't congest
the Sync queue that feeds the collectives; per-core reduction rows
collect in a staging tile written with one transposed DMA at the end.
"""

import os
import sys

import numpy as np

for _p in ("/opt/trn_rl_repo", os.path.expanduser("~/.axon_site/_ro/trn_rl_repo")):
    if os.path.isdir(_p) and _p not in sys.path:
        sys.path.insert(0, _p)

import concourse.bacc as bacc
import concourse.bass as bass
import concourse.mybir as mybir
import concourse.tile as tile
from concourse import bass_utils
from concourse.masks import make_identity

N, E, G, V, R = 4096, 512, 256, 50000, 1000
NCORES = 8
NL = N // NCORES  # 512 rows per core
P = 128
NE = E // P   # 4 e-chunks
NH = G // P   # 2 hidden chunks for the g-LSTM
NMB = N // P  # 32 m-blocks
EPS = 1e-8

F32 = mybir.dt.float32
BF16 = mybir.dt.bfloat16
F8 = mybir.dt.float8e4
I32 = mybir.dt.int32
AF = mybir.ActivationFunctionType
ALU = mybir.AluOpType
DR = mybir.MatmulPerfMode.DoubleRow


def _gather8(nc, pools, emb8, idx_dram, ident8, dstT8):
    """Gather NL fp8 embedding rows and transpose into dstT8 (P, NE, NL)."""
    ip, rp, pt = pools["idx"], pools["raw"], pools["pt"]
    for t in range(NL // P):
        idx_t = ip.tile([P, 1], I32, tag="idx", bufs=8, name="idx_t")
        nc.sync.dma_start(out=idx_t[:], in_=idx_dram[t * P:(t + 1) * P, :])
        raw = rp.tile([P, E], F8, tag="raw", bufs=4, name="raw")
        nc.gpsimd.indirect_dma_start(
            out=raw[:], out_offset=None, in_=emb8[:],
            in_offset=bass.IndirectOffsetOnAxis(ap=idx_t[:, :1], axis=0))
        # fp8 transpose outputs must land with element step 2 (HW rule)
        ptile = pt.tile([P, NE, P, 2], F8, tag="pt", bufs=2, name="ptile")
        for et in range(NE):
            nc.tensor.transpose(
                out=ptile[:, et, :, 0], in_=raw[:, et * P:(et + 1) * P],
                identity=ident8[:])
        nc.vector.tensor_copy(
            out=dstT8[:, :, t * P:(t + 1) * P], in_=ptile[:, :, :, 0])


def _xgates(nc, pg, xT8, W_sb, gates, hc, s0=0):
    """Stream x-side gate matmuls, moving-tensor-outer for PE locality.

    Returns {gate: psum tile [P, 2, NL]} covering feature chunks
    (g*hc + s0, g*hc + s0 + 1). No bias (it rides the activation).
    """
    ps = {g: pg.tile([P, 2, NL], F32, tag="pg2", bufs=3, name="ps_gate")
          for g in gates}
    for i in range(NE // 2):
        for g in gates:
            for s in range(2):
                jc = g * hc + s0 + s
                js = slice(jc * P, (jc + 1) * P)
                nc.tensor.matmul(
                    ps[g][:, s, :], W_sb[:, 2 * i:2 * i + 2, js],
                    xT8[:, 2 * i:2 * i + 2, :],
                    start=(i == 0), stop=(i == NE // 2 - 1),
                    perf_mode=DR, skip_group_check=True)
    return ps


def build_program():
    nc = bacc.Bacc("TRN2", target_bir_lowering=False, debug=False,
                   enable_asserts=False, num_devices=NCORES)
    dram = lambda name, shape, dt=F32, kind="ExternalInput": \
        nc.dram_tensor(name, shape, dt, kind=kind).ap()

    emb8 = dram("emb8", [V, E], F8)
    idx_x = dram("idx_x", [NL, 1], I32)
    idx_e0 = dram("idx_e0", [NL, 1], I32)
    idx_e1 = dram("idx_e1", [NL, 1], I32)
    wgf = dram("wgf", [P, NE, 4 * G], F8)
    wgr = dram("wgr", [P, NE, 4 * G], F8)
    ugf = dram("ugf", [P, NH, 4 * G], F8)
    ugr = dram("ugr", [P, NH, 4 * G], F8)
    wf = dram("wf", [P, NE, 4 * E], F8)
    bgf = dram("bgf", [P, 4 * G // P])
    bgr = dram("bgr", [P, 4 * G // P])
    bf = dram("bf", [P, 4 * E // P])
    out = dram("out", [8, E], kind="ExternalOutput")

    with tile.TileContext(nc) as tc:
        _emit(tc, locals())
    nc.compile()
    return nc


def _emit(tc, T):
    nc = tc.nc
    rg = [list(range(NCORES))]
    from contextlib import ExitStack
    ctx = ExitStack()
    with ctx:
        glob = ctx.enter_context(tc.tile_pool(name="glob", bufs=1))
        dramp = ctx.enter_context(tc.tile_pool(name="dramp", bufs=1,
                                               space="DRAM"))

        identf = glob.tile([P, P], F32)
        make_identity(nc, identf)
        ident8 = glob.tile([P, P], F8)
        nc.vector.tensor_copy(out=ident8[:], in_=identf[:])
        # staging for the 8 per-core reduction rows; one DMA at the end
        stag = glob.tile([P, 8, NE], F32)

        # collective bounce buffers (declared wide: fewer descriptor rows)
        ag1_src_w = dramp.tile([P, 4 * NL], F8)
        ag1_dst_w = dramp.tile([NCORES * P, 4 * NL], F8, addr_space="Shared")
        ag1s = ag1_src_w.rearrange("a (r b) -> (a r) b", r=4)    # (E, NL)
        ag1d = ag1_dst_w.rearrange("a (r b) -> (a r) b", r=4)    # (8E, NL)
        ag2_src_w = dramp.tile([2 * NL // 4, 4 * E], F8)
        ag2_dst_w = dramp.tile([NCORES * 2 * NL // 4, 4 * E], F8,
                               addr_space="Shared")
        ag2s = ag2_src_w.rearrange("a (r b) -> (a r) b", r=4)    # (2NL, E)
        ag2d = ag2_dst_w.rearrange("a (r b) -> (a r) b", r=4)    # (8*2NL, E)

        # fp8 activations (g_out lives fp8 end-to-end)
        g08 = glob.tile([P, NE, NL], F8)   # [hf0 | hr0]
        g18 = glob.tile([P, NE, NL], F8)   # [hf1 | hr1]
        dgT8 = glob.tile([P, NE, NL], F8)
        A0T = glob.tile([P, NMB, NL], F8)

        with tc.tile_pool(name="wpool", bufs=1) as wp, \
             tc.tile_pool(name="acts", bufs=1) as ap_, \
             tc.tile_pool(name="gates", bufs=1) as gp, \
             tc.tile_pool(name="tmp", bufs=1) as tp, \
             tc.tile_pool(name="idx", bufs=1) as ip, \
             tc.tile_pool(name="raw", bufs=1) as rp, \
             tc.tile_pool(name="pg", bufs=1, space="PSUM") as pgp, \
             tc.tile_pool(name="pt", bufs=1, space="PSUM") as ptp:
            pools = {"idx": ip, "raw": rp, "pt": ptp}

            # ---- weights on the Act hwdge queue (early, uncontended) ----
            w_sb = {}
            for nm, kt in (("wf", NE), ("wgf", NE), ("wgr", NE),
                           ("ugf", NH), ("ugr", NH)):
                hw = 4 * (E if nm == "wf" else G)
                w_sb[nm] = wp.tile([P, kt, hw], F8, name=nm + "_sb")
                nc.scalar.dma_start(out=w_sb[nm][:], in_=T[nm][:])
            for nm, hw in (("bf", 16), ("bgf", 8), ("bgr", 8)):
                w_sb[nm] = wp.tile([P, hw], F32, name=nm + "_sb")
                nc.scalar.dma_start(out=w_sb[nm][:], in_=T[nm][:])

            # ---- gathers (x first: the f-LSTM feeds AG1) ----
            xT8 = ap_.tile([P, NE, NL], F8)
            e0T8 = ap_.tile([P, NE, NL], F8)
            e1T8 = ap_.tile([P, NE, NL], F8)
            _gather8(nc, pools, T["emb8"], T["idx_x"], ident8, xT8)
            _gather8(nc, pools, T["emb8"], T["idx_e0"], ident8, e0T8)
            _gather8(nc, pools, T["emb8"], T["idx_e1"], ident8, e1T8)

            # ---- one PE stream: all x-side gate matmuls ----
            ZG = (0, 2, 3)  # i, g, o (forget unused with zero state)
            fps = [_xgates(nc, pgp, xT8, w_sb["wf"], ZG, NE, s0=2 * h)
                   for h in range(2)]
            c1ps = _xgates(nc, pgp, e0T8, w_sb["wgf"], ZG, NH)
            c2ps = _xgates(nc, pgp, e1T8, w_sb["wgr"], ZG, NH)
            c3ps, c4ps = {}, {}
            for gpair in ((0, 1), (2, 3)):
                c3ps.update(_xgates(nc, pgp, e1T8, w_sb["wgf"], gpair, NH))
                c4ps.update(_xgates(nc, pgp, e0T8, w_sb["wgr"], gpair, NH))
            # spill recurrent cells' x-parts to bf16 (psum is scarce)
            xp3 = ap_.tile([P, 4, 2, NL], BF16, name="xp3")
            xp4 = ap_.tile([P, 4, 2, NL], BF16, name="xp4")
            for g in range(4):
                if g % 2:
                    nc.scalar.copy(out=xp3[:, g], in_=c3ps[g][:])
                    nc.scalar.copy(out=xp4[:, g], in_=c4ps[g][:])
                else:
                    nc.vector.tensor_copy(out=xp3[:, g], in_=c3ps[g][:])
                    nc.vector.tensor_copy(out=xp4[:, g], in_=c4ps[g][:])

            def zcell(ps, b_sb, hc, s0, h_out, c_out):
                """Zero-state cell tail: per-chunk acts (+bias), c/h chain."""
                gb = {}
                for g, func in ((0, AF.Sigmoid), (2, AF.Tanh),
                                (3, AF.Sigmoid)):
                    t = gp.tile([P, 2, NL], F32, tag=f"zg{g}", bufs=2,
                                name=f"zg{g}")
                    for s in range(2):
                        jc = g * hc + s0 + s
                        nc.scalar.activation(
                            out=t[:, s, :], in_=ps[g][:, s, :], func=func,
                            bias=b_sb[:, jc:jc + 1])
                    gb[g] = t
                nc.gpsimd.tensor_mul(c_out[:], gb[0][:], gb[2][:])
                tc_ = tp.tile([P, 2, NL], F32, tag="tanhc", bufs=2,
                              name="tanhc")
                nc.scalar.activation(out=tc_[:], in_=c_out[:], func=AF.Tanh)
                nc.vector.tensor_mul(h_out[:], gb[3][:], tc_[:])

            # f-cell first (feeds AG1), then c1/c2
            fh8 = ap_.tile([P, NE, NL], F8, name="fh8")
            with tc.high_priority():
                for h in range(2):
                    hs = slice(2 * h, 2 * h + 2)
                    cf_ = tp.tile([P, 2, NL], F32, tag="cf", bufs=2,
                                  name="cf")
                    hf_ = tp.tile([P, 2, NL], F32, tag="hf", bufs=2,
                                  name="hf")
                    zcell(fps[h], w_sb["bf"], NE, 2 * h, hf_, cf_)
                    nc.vector.tensor_add(fh8[:, hs, :], hf_[:], xT8[:, hs, :])
                nc.sync.dma_start(
                    out=ag1s[:].rearrange("(et p) n -> p et n", p=P),
                    in_=fh8[:])
                nc.gpsimd.collective_compute(
                    "AllGather", ALU.bypass, replica_groups=rg,
                    ins=[ag1_src_w[:].opt()], outs=[ag1_dst_w[:].opt()])

            cfT = ap_.tile([P, NH, NL], F32, name="cfT")
            crT = ap_.tile([P, NH, NL], F32, name="crT")
            zcell(c1ps, w_sb["bgf"], NH, 0, g08[:, 0:NH, :], cfT)    # hf0
            zcell(c2ps, w_sb["bgr"], NH, 0, g18[:, NH:NE, :], crT)   # hr1

            def rcell(xp, U_sb, b_sb, hprev8, cprev, h_out):
                """Recurrent cell tail: U@h + spilled x-part, acts, c/h."""
                ups = {g: pgp.tile([P, 2, NL], F32, tag="pg2", bufs=3,
                                   name="ups") for g in range(4)}
                for g in range(4):
                    for s in range(2):
                        nc.tensor.matmul(
                            ups[g][:, s, :],
                            U_sb[:, :, (g * NH + s) * P:(g * NH + s + 1) * P],
                            hprev8[:], start=True, stop=True, perf_mode=DR,
                            skip_group_check=True)
                gb = []
                for g in range(4):
                    gt = gp.tile([P, 2, NL], F32, tag=f"rg{g}", bufs=2,
                                 name=f"rg{g}")
                    nc.vector.tensor_add(gt[:], ups[g][:], xp[:, g])
                    func = AF.Tanh if g == 2 else AF.Sigmoid
                    for s in range(2):
                        nc.scalar.activation(
                            out=gt[:, s, :], in_=gt[:, s, :], func=func,
                            bias=b_sb[:, g * NH + s:g * NH + s + 1])
                    gb.append(gt)
                ig = tp.tile([P, 2, NL], F32, tag="ig", bufs=2, name="ig")
                nc.gpsimd.tensor_mul(ig[:], gb[0][:], gb[2][:])
                cc = tp.tile([P, 2, NL], F32, tag="cc", bufs=2, name="cc")
                nc.gpsimd.tensor_mul(cc[:], gb[1][:], cprev[:])
                nc.gpsimd.tensor_add(cc[:], cc[:], ig[:])
                tc_ = tp.tile([P, 2, NL], F32, tag="tanhc", bufs=2,
                              name="tanhc")
                nc.scalar.activation(out=tc_[:], in_=cc[:], func=AF.Tanh)
                nc.vector.tensor_mul(h_out[:], gb[3][:], tc_[:])

            rcell(xp3, w_sb["ugf"], w_sb["bgf"], g08[:, 0:NH, :], cfT,
                  g18[:, 0:NH, :])   # hf1
            rcell(xp4, w_sb["ugr"], w_sb["bgr"], g18[:, NH:NE, :], crT,
                  g08[:, NH:NE, :])  # hr0

            nc.vector.tensor_sub(dgT8[:], g08[:], g18[:])

            # transpose g0/g1 to n-major and fire AG2
            with tc.high_priority():
                for src8, row0 in ((g08, 0), (g18, NL)):
                    for nt in range(NL // P):
                        ptile = ptp.tile([P, NE, P, 2], F8, tag="pt", bufs=2,
                                         name="ptg")
                        for et in range(NE):
                            nc.tensor.transpose(
                                out=ptile[:, et, :, 0],
                                in_=src8[:, et, nt * P:(nt + 1) * P],
                                identity=ident8[:])
                        stile = tp.tile([P, E], F8, tag="tps", bufs=3,
                                        name="stile")
                        nc.vector.tensor_copy(
                            out=stile[:].rearrange("p (et q) -> p et q", q=P),
                            in_=ptile[:, :, :, 0])
                        nc.sync.dma_start(
                            out=ag2s[row0 + nt * P:row0 + (nt + 1) * P, :],
                            in_=stile[:])
                nc.gpsimd.collective_compute(
                    "AllGather", ALU.bypass, replica_groups=rg,
                    ins=[ag2_src_w[:].opt()], outs=[ag2_dst_w[:].opt()])

        # ---- phase C: fh preload (Act hwdge queue) + D1 ----
        dp = ctx.enter_context(tc.tile_pool(name="dpool", bufs=1))
        fhAll = dp.tile([P, NCORES, NE, NL], F8)
        for k in range(NCORES):
            nc.scalar.dma_start(
                out=fhAll[:, k, :, :],
                in_=ag1d[k * E:(k + 1) * E, :].rearrange(
                    "(et p) n -> p et n", p=P))

        with tc.tile_pool(name="pd", bufs=1, space="PSUM") as pdp:
            for k in range(NCORES):
                for cp in range(2):
                    pd2 = pdp.tile([P, 2, NL], F32, tag="pd", bufs=3,
                                   name="pd2")
                    for cc in range(2):
                        c = 2 * cp + cc
                        for i in range(NE // 2):
                            nc.tensor.matmul(
                                pd2[:, cc, :],
                                fhAll[:, k, 2 * i:2 * i + 2,
                                      c * P:(c + 1) * P],
                                dgT8[:, 2 * i:2 * i + 2, :],
                                start=(i == 0), stop=(i == NE // 2 - 1),
                                perf_mode=DR)
                    mb = 4 * k + 2 * cp
                    nc.scalar.activation(
                        out=A0T[:, mb:mb + 2, :], in_=pd2[:], func=AF.Sigmoid)

        # early local reductions (DVE slots into the collective-wire gap):
        # rows 2 (sg0), 3 (sg1), 7 (S1c = local colsum of g1)
        with tc.tile_pool(name="ered", bufs=1) as ep:
            for row, gT in ((2, g08), (3, g18)):
                for et in range(NE):
                    scr = ep.tile([P, NL], F32, tag="scr0", bufs=2,
                                  name="scr0")
                    nc.vector.tensor_mul(scr[:], gT[:, et, :], gT[:, et, :])
                    nc.vector.reduce_sum(out=stag[:, row, et:et + 1],
                                         in_=scr[:],
                                         axis=mybir.AxisListType.X)
            for et in range(NE):
                nc.vector.reduce_sum(out=stag[:, 7, et:et + 1],
                                     in_=g18[:, et, :],
                                     axis=mybir.AxisListType.X)

        # gAll preload (Act hwdge queue; emitted after D1 so D1's
        # sigmoids aren't queued behind an AG2-gated trigger)
        gAll = dp.tile([P, 2 * NMB, E], F8)
        for k in range(NCORES):
            nc.scalar.dma_start(
                out=gAll[:, 4 * k:4 * k + 4, :],
                in_=ag2d[k * 2 * NL:k * 2 * NL + NL, :].rearrange(
                    "(c p) e -> p c e", p=P))
            nc.scalar.dma_start(
                out=gAll[:, NMB + 4 * k:NMB + 4 * k + 4, :],
                in_=ag2d[k * 2 * NL + NL:(k + 1) * 2 * NL, :].rearrange(
                    "(c p) e -> p c e", p=P))

        # ---- phase D: r0 = a0@g0, q1 = a0@g1 (PSUM accum over m) ----
        with tc.tile_pool(name="pr", bufs=1, space="PSUM") as prp, \
             tc.tile_pool(name="fin", bufs=1) as fin:
            r0p = [prp.tile([P, NL], F32, tag=f"r0_{et}", name=f"r0_{et}")
                   for et in range(NE)]
            q1p = [prp.tile([P, NL], F32, tag=f"q1_{et}", name=f"q1_{et}")
                   for et in range(NE)]
            for t in range(NMB // 2):
                a0sl = A0T[:, 2 * t:2 * t + 2, :]
                for et in range(NE):
                    es = slice(et * P, (et + 1) * P)
                    nc.tensor.matmul(
                        r0p[et][:], gAll[:, 2 * t:2 * t + 2, es], a0sl,
                        start=(t == 0), stop=(t == NMB // 2 - 1),
                        perf_mode=DR)
                    nc.tensor.matmul(
                        q1p[et][:], gAll[:, NMB + 2 * t:NMB + 2 * t + 2, es],
                        a0sl,
                        start=(t == 0), stop=(t == NMB // 2 - 1),
                        perf_mode=DR)

            # ---- phase E: reductions over local n into staging ----
            # rows: 0=dot0 1=sr0 4=A(sum q1) 5=B(sum q1^2) 6=C(sum q1 g1)
            for et in range(NE):
                scr2 = fin.tile([P, NL], F32, tag="scr2", bufs=2, name="scr2")
                nc.vector.tensor_mul(scr2[:], r0p[et][:], g08[:, et, :])
                nc.vector.reduce_sum(out=stag[:, 0, et:et + 1], in_=scr2[:],
                                     axis=mybir.AxisListType.X)
                scr3 = fin.tile([P, NL], F32, tag="scr2", bufs=2, name="scr3")
                nc.vector.tensor_mul(scr3[:], q1p[et][:], g18[:, et, :])
                nc.vector.reduce_sum(out=stag[:, 6, et:et + 1], in_=scr3[:],
                                     axis=mybir.AxisListType.X)
                junk = fin.tile([P, NL], F32, tag="junk", bufs=4, name="junk")
                nc.scalar.activation(out=junk[:], in_=r0p[et][:],
                                     func=AF.Square,
                                     accum_out=stag[:, 1, et:et + 1])
                junk2 = fin.tile([P, NL], F32, tag="junk", bufs=4,
                                 name="junk2")
                nc.scalar.activation(out=junk2[:], in_=q1p[et][:],
                                     func=AF.Square,
                                     accum_out=stag[:, 5, et:et + 1])
                junk3 = fin.tile([P, NL], F32, tag="junk", bufs=4,
                                 name="junk3")
                nc.scalar.activation(out=junk3[:], in_=q1p[et][:],
                                     func=AF.Identity,
                                     accum_out=stag[:, 4, et:et + 1])

        # single transposed output DMA: stag [P, 8, 4] -> out [8, E]
        with tc.tile_pool(name="po", bufs=1, space="PSUM") as pop, \
             tc.tile_pool(name="fo", bufs=1) as fop:
            ot = pop.tile([32, P], F32)
            nc.tensor.transpose(out=ot[:],
                                in_=stag[:].rearrange("p r e -> p (r e)"),
                                identity=identf[:])
            os_ = fop.tile([32, P], F32)
            nc.vector.tensor_copy(out=os_[:], in_=ot[:])
            nc.sync.dma_start(
                out=T["out"][:].rearrange("r (et p) -> (r et) p", p=P),
                in_=os_[:])


_PROGRAM = None


def _get_program():
    global _PROGRAM
    if _PROGRAM is None:
        _PROGRAM = build_program()
    return _PROGRAM


def _f8np():
    return mybir.dt.np(F8)


def _prep_w(w):
    """(4H, E_in) torch-layout weight -> fp8 lhsT tiles [p, kt, 4H]."""
    wt = np.asarray(w, np.float32).T  # (E_in, 4H)
    e_in, fourh = wt.shape
    t = np.ascontiguousarray(
        wt.reshape(e_in // P, P, fourh).transpose(1, 0, 2))
    return t.astype(_f8np())


def _prep_b(b1, b2):
    """Summed bias laid out [P, n_chunks] f32 (per-chunk activation bias)."""
    s = np.asarray(b1, np.float32) + np.asarray(b2, np.float32)
    return np.ascontiguousarray(s.reshape(-1, P).T)


def run_device(inputs, trace=False):
    """Shard inputs, run the 8-core SPMD program, return results."""
    nc = _get_program()
    emb8 = np.asarray(inputs["embedding"], np.float32).astype(_f8np())
    iq = np.asarray(inputs["input"]).astype(np.int32).reshape(N, 1)
    ie = np.asarray(inputs["set_inputs"]).astype(np.int32)
    shared = {
        "emb8": np.ascontiguousarray(emb8),
        "wgf": _prep_w(inputs["wih_gf"]), "wgr": _prep_w(inputs["wih_gr"]),
        "ugf": _prep_w(inputs["whh_gf"]), "ugr": _prep_w(inputs["whh_gr"]),
        "wf": _prep_w(inputs["wih_f"]),
        "bgf": _prep_b(inputs["bih_gf"], inputs["bhh_gf"]),
        "bgr": _prep_b(inputs["bih_gr"], inputs["bhh_gr"]),
        "bf": _prep_b(inputs["bih_f"], inputs["bhh_f"]),
    }
    in_maps = []
    for k in range(NCORES):
        sl = slice(k * NL, (k + 1) * NL)
        m = dict(shared)
        m["idx_x"] = np.ascontiguousarray(iq[sl])
        m["idx_e0"] = np.ascontiguousarray(ie[0, sl].reshape(NL, 1))
        m["idx_e1"] = np.ascontiguousarray(ie[1, sl].reshape(NL, 1))
        in_maps.append(m)
    res = bass_utils.run_bass_kernel_spmd(
        nc, in_maps, core_ids=list(range(NCORES)), trace=trace)
    return res


def kernel(**inputs):
    res = run_device(inputs)
    return host_tail(res, inputs)


def host_tail(res, inputs):
    acc = np.zeros((8, E), np.float64)
    for r in res.results:
        acc += r["out"]
    dot0, sr0, sg0, sg1, A, B, C, S1 = acc
    dot1 = S1 * S1 - C
    sr1 = N * S1 * S1 - 2.0 * S1 * A + B
    dot = np.stack([dot0, dot1])
    sr = np.stack([sr0, sr1])
    sg = np.stack([sg0, sg1])
    nr = np.maximum(np.sqrt(sr), EPS)
    ng = np.maximum(np.sqrt(sg), EPS)
    cos = dot / (nr * ng)
    kern = cos / np.exp(cos).sum()
    w_out = np.asarray(inputs["w_out"], np.float64)
    b_out = np.asarray(inputs["b_out"], np.float64)
    k2 = kern @ w_out.T + b_out                  # (2, R)
    s = k2.sum(axis=1)                           # (2,)
    labels = np.asarray(inputs["set_labels"], np.float64)
    o = s[0] * labels[0] + s[1] * labels[1]      # (R,)
    o = np.exp(o - o.max())
    o /= o.sum()
    return o.astype(np.float32)
